# revision 6
# baseline (speedup 1.0000x reference)
"""Grouped Conv2D (32 groups of 8->8 ch, 3x3, SAME) on 8 trn2 NeuronCores.

Strategy:
  - Data-parallel over batch: 32 images / 8 cores = 4 images per core.
  - 2x2 PE-array tiling (4 concurrent 64x64 tiles): the 2 column-groups
    stream the same 8-row strip of two different images; the 2 row-tiles
    of a column share its stream and hold block-diagonal weights for 8
    groups (8 x [8ic x 8oc]) -> 512 useful MACs per streamed column, the
    max for this group structure. ~8 matmuls in flight (4 tiles x 2
    pipelined) saturates the per-tile streaming rate.
  - Per batch (strip s of images a,b): 9 taps x 4 tile-matmuls accumulate
    in PSUM (bank r, partitions 64c); an engine copy (scalar/vector
    alternating) evacuates to a per-image-pair bf16 staging tile keyed by
    strip, so each (image, chunk) leaves SBUF as ONE DMA with 6.3KB
    contiguous runs per channel (fast descriptors, no scatter tail).
  - bf16 in/out over HBM (host casts), fp32 PSUM accumulate.
"""

import sys

import numpy as np

if "/opt/trn_rl_repo" not in sys.path:
    sys.path.insert(0, "/opt/trn_rl_repo")

import ml_dtypes

B, C, H, W = 32, 256, 56, 56
KK = 3
GROUPS = 32
CPG = 8  # in- and out-channels per group
N_CORES = 8
BPC = B // N_CORES  # images per core
HP, WP = H + 2, W + 2  # padded image
NCHUNK = 2  # 256 channels = 2 x 128 partitions
STRIP = 8  # output rows per instance (8*56=448 <= 512 fp32/bank)
NSTRIP = H // STRIP  # 7
NTAP = KK * KK
NCOL = 2  # concurrent column-groups (one image each)
NROW = 2  # row-tiles per column (8 groups each)
GPT = 8  # groups per tile
WU_ROUNDS = 24  # PE warm-up waves (data-independent, start immediately)


def _pack_weights(w: np.ndarray) -> np.ndarray:
    """[256, 8, 3, 3] fp32 -> [128 pc, 2 ck, 9 tap, 64] bf16.

    wpk[64r + 8j + ic, ck, 3*th+tw, 8j + oc] = w[128ck + 64r + 8j + oc, ic, th, tw]
    """
    wr = w.reshape(NCHUNK, NROW, GPT, CPG, CPG, KK, KK)  # ck, r, j, oc, ic, th, tw
    wpk = np.zeros((NROW, GPT, CPG, NCHUNK, NTAP, GPT, CPG), dtype=np.float32)
    for j in range(GPT):
        # [ck, r, oc, ic, th, tw] -> [r, ic, ck, (th tw), oc]
        blk = wr[:, :, j].transpose(1, 3, 0, 4, 5, 2).reshape(NROW, CPG, NCHUNK, NTAP, CPG)
        wpk[:, j, :, :, :, j, :] = blk
    return wpk.reshape(128, NCHUNK, NTAP, 64).astype(ml_dtypes.bfloat16)


def _build_bass():
    import concourse.tile as tile
    from concourse import bacc, mybir

    nc = bacc.Bacc()
    xs = nc.dram_tensor(
        "xs", [BPC, C, HP, WP], mybir.dt.bfloat16, kind="ExternalInput"
    )
    wpk = nc.dram_tensor(
        "wpk", [128, NCHUNK, NTAP, 64], mybir.dt.bfloat16, kind="ExternalInput"
    )
    out = nc.dram_tensor(
        "out", [BPC, C, H, W], mybir.dt.bfloat16, kind="ExternalOutput"
    )

    with tile.TileContext(nc) as tc:
        with (
            tc.tile_pool(name="singles", bufs=1) as singles,
            tc.tile_pool(name="xpad_pool", bufs=4) as xpad_pool,
            tc.tile_pool(name="ot_pool", bufs=2) as ot_pool,
            tc.tile_pool(name="psum_pool", bufs=4, space="PSUM") as psum_pool,
        ):
            # PE warm-up in the same 2x2 tiled mode as the real matmuls
            # (mode switches drain the array), on a memset scratch tile so
            # it needs no input data and starts immediately, covering the
            # HAM clock ramp while the first inputs and weights stream in.
            wu_src = singles.tile([128, 512], mybir.dt.bfloat16)
            nc.vector.memset(wu_src[:], 0.0)
            wu = psum_pool.tile([128, NROW, 512], mybir.dt.float32, name="ps")
            for _ in range(WU_ROUNDS):
                for cg in range(NCOL):
                    for r in range(NROW):
                        nc.tensor.matmul(
                            wu[64 * cg : 64 * cg + 64, r, : STRIP * W],
                            lhsT=wu_src[64 * r : 64 * r + 64, :64],
                            rhs=wu_src[64 * r : 64 * r + 64, :448],
                            start=True,
                            stop=True,
                            tile_position=(64 * r, 64 * cg),
                        )

            w_sb = singles.tile([128, NCHUNK, NTAP, 64], mybir.dt.bfloat16)
            nc.sync.dma_start(out=w_sb[:], in_=wpk[:])

            # input tiles, issued in consumption order (ck-major, img minor);
            # bufs=4 keeps the current image pair + the next one in flight
            # and throttles SW-DGE round-robin. The first two tiles (both
            # images of batch 0) are split at the rows needed by strips 0-1,
            # first parts on HW-DGE for a fast path to the first matmul.
            SPLIT = 2 * STRIP + 2  # 18
            xpads = {}
            for ck in range(NCHUNK):
                for img in range(BPC):
                    xp = xpad_pool.tile([128, HP, WP], mybir.dt.bfloat16, name="xpad")
                    src = xs[img, ck * 128 : (ck + 1) * 128]
                    if ck == 0 and img < 2:
                        nc.sync.dma_start(out=xp[:, :SPLIT, :], in_=src[:, :SPLIT, :])
                        nc.gpsimd.dma_start(out=xp[:, SPLIT:, :], in_=src[:, SPLIT:, :])
                    else:
                        nc.gpsimd.dma_start(out=xp[:], in_=src)
                    xpads[(ck, img)] = xp

            # 8 groups = 2 chunks x 2 image-pairs; 7 strip-batches per group
            n_batch = 0
            for ck in range(NCHUNK):
                for pair in ((0, 1), (2, 3)):
                    ot = ot_pool.tile(
                        [128, NROW, NSTRIP, STRIP, W], mybir.dt.bfloat16, name="otg"
                    )
                    for s in range(NSTRIP):
                        ps = psum_pool.tile(
                            [128, NROW, 512], mybir.dt.float32, name="ps"
                        )
                        for t in range(NTAP):
                            th, tw = divmod(t, KK)
                            for cg, img in enumerate(pair):
                                for r in range(NROW):
                                    nc.tensor.matmul(
                                        ps[64 * cg : 64 * cg + 64, r, : STRIP * W],
                                        lhsT=w_sb[64 * r : 64 * r + 64, ck, t, :],
                                        rhs=xpads[(ck, img)][
                                            64 * r : 64 * r + 64,
                                            s * STRIP + th : s * STRIP + th + STRIP,
                                            tw : tw + W,
                                        ],
                                        start=(t == 0),
                                        stop=(t == NTAP - 1),
                                        tile_position=(64 * r, 64 * cg),
                                    )
                        copy = (
                            nc.scalar.copy if n_batch % 2 == 0 else nc.vector.tensor_copy
                        )
                        copy(out=ot[:, :, s], in_=ps[:, :, : STRIP * W])
                        n_batch += 1
                    for cg, img in enumerate(pair):
                        dst = out[img, ck * 128 : (ck + 1) * 128].rearrange(
                            "(r p) h w -> p r h w", r=NROW
                        )
                        nc.sync.dma_start(out=dst, in_=ot[64 * cg : 64 * cg + 64])
    nc.finalize()
    return nc


_CACHE = {}


def kernel(x, w, trace=False):
    from concourse.bass_utils import run_bass_kernel_spmd

    x = np.asarray(x)
    w = np.ascontiguousarray(np.asarray(w), dtype=np.float32)

    if "nc" not in _CACHE:
        _CACHE["nc"] = _build_bass()
    nc = _CACHE["nc"]

    xbf = np.zeros((B, C, HP, WP), dtype=ml_dtypes.bfloat16)
    xbf[:, :, 1 : H + 1, 1 : W + 1] = x.astype(ml_dtypes.bfloat16)
    wpk = _pack_weights(w)
    in_maps = [
        {"xs": np.ascontiguousarray(xbf[i * BPC : (i + 1) * BPC]), "wpk": wpk}
        for i in range(N_CORES)
    ]
    res = run_bass_kernel_spmd(
        nc, in_maps, core_ids=list(range(N_CORES)), trace=trace
    )
    outs = np.concatenate([res.results[i]["out"] for i in range(N_CORES)], axis=0)
    if trace:
        kernel.last_result = res
    return outs.astype(np.float32)
